# revision 16
# baseline (speedup 1.0000x reference)
"""LowRankKernel for 8x TRN2 NeuronCores (Bass/Tile, SPMD) — wire-optimized.

Math (reference):
  psi = MLP_psi(coords)  [H,W,R,C_IN]   (erf GELU, HID=256)
  phi = MLP_phi(coords)  [H,W,R,C_OUT]
  l2[b,r]   = sum_{h,w,i} psi[h,w,r,i] * v[b,i,h,w] * dx^2
  u[b,o,h,w] = sum_r l2[b,r] * phi[h,w,r,o]

Distribution: spatial sharding over H (16 rows / core). Only the tiny
[64,64] l2 tensor is AllReduced; w2 weights are shipped *sharded* (1/8
each) and reassembled on-device with an AllGather over the fast
device-to-device links.

Under axon the host<->device tunnel (~50 MB/s) dominates wall time, so
this version minimizes wire bytes:
  - v is shipped as 10-bit linear fixed-point: an int8 low-byte plane
    plus a crumb-packed (2-bit) high plane (1.25 B/elem) in a single
    uint8 tensor, unpacked to bf16 on-device by the DVE. Quantization
    noise is ~2x bf16's, far below the int8 output floor.
  - u is shipped as int8 with per-(core, batch) scales computed on
    device (two-pass over a DRAM spill with per-partition abs-max);
    host decodes q/inv. Quantization step is <= max|u|/126.
  - MLP weights ship as bf16 shards (0.5 MB/core instead of 8 MB/core).
  - The executor is a cached jax.jit of the same _bass_exec_p custom
    call that bass_utils.run_bass_kernel_spmd lowers to under axon,
    with the NEFF's donated output buffers kept device-resident across
    calls (no per-call zero-buffer upload, no per-call retrace).

Per-core pipeline:
  0: DMA misc params; AllGather w2 shards -> full [512,4096] in DRAM;
     DMA into per-hid-half SBUF tiles (bf16).
  A: coords -> H_T (hidden, transposed [hid, p]) via matmul + erf-GELU
     (ACT), output bf16.
  B: per p-tile (128 grid points): unpack v slab (lo + hi nibbles ->
     bf16), psi tile [p, (i-major, r)] = H_T.T @ W2psi + bias (bf16),
     then 64 accumulating matmuls (bf16) against v slabs -> l2^T [r,b]
     in PSUM. Scale by sv*dx^2 (per-call scalar shipped in misc).
  AllReduce l2 (16KB) across 8 cores.
  C: per c'-tile: phi^T tile [(o,r), p] = W2phi.T @ H_T + bias (f32r),
     u[b, (o,p)] = l2^T.T @ phi_slice (f32r) -> f32 spill to DRAM with
     per-partition (= per-b) running abs-max; second pass rescales to
     int8 (trunc with +-0.5 pre-offset == round-to-nearest) -> u_out.
"""
import sys
if '/opt/trn_rl_repo' not in sys.path:
    sys.path.insert(0, '/opt/trn_rl_repo')

import numpy as np
import ml_dtypes

import concourse.bass as bass
import concourse.mybir as mybir
from concourse import tile

F32 = mybir.dt.float32
F32R = mybir.dt.float32r
BF16 = mybir.dt.bfloat16
U8 = mybir.dt.uint8
I8 = mybir.dt.int8
AF = mybir.ActivationFunctionType
ALU = mybir.AluOpType

B, C_IN, C_OUT, H, W, RANK, HID = 64, 64, 64, 128, 128, 64, 256
N_CORES = 8
HL = H // N_CORES           # 16 h-rows per core
P = HL * W                  # 2048 grid points per core
NPT = P // 128              # 16 p-tiles per core
DX = 1.0 / (W - 1)
NC2 = RANK * C_IN           # 4096 columns of the MLP2 output

# misc param-block offsets (f32 elements)
OFF_COORDS = 0                       # [2, P] flattened
OFF_W1PSI = OFF_COORDS + 2 * P       # [2, HID]
OFF_W1PHI = OFF_W1PSI + 2 * HID
OFF_B1PSI = OFF_W1PHI + 2 * HID      # [128, 2] flattened
OFF_B1PHI = OFF_B1PSI + 256
OFF_B2PSI = OFF_B1PHI + 256          # [4096] i-major
OFF_B2PHI = OFF_B2PSI + NC2          # [128, 32] flattened
OFF_SVDX2 = OFF_B2PHI + NC2          # [1] = s_v * dx^2
MISC_LEN = OFF_SVDX2 + 1

_CACHE = {}


def _split_multi_waits(nc):
    """This walrus build only supports one sync-wait command per instruction.
    Move extra waits onto standalone single-wait EventSemaphore instructions
    placed immediately before, on the same engine (same semantics)."""
    n_new = 0
    for fn in nc.m.functions:
        for bb in fn.blocks:
            new_list = []
            changed = False
            for inst in bb.instructions:
                si = inst.sync_info
                if si is not None and len(si.on_wait) > 1:
                    changed = True
                    waits = list(si.on_wait)
                    for w in waits[:-1]:
                        n_new += 1
                        ev = mybir.InstEventSemaphore(
                            name=f"{inst.name}-presplit{n_new}",
                            engine=inst.engine, ins=[], outs=[],
                            sync_info=mybir.SyncInfo(on_wait=[w], on_update=[]),
                        )
                        new_list.append(ev)
                    inst.sync_info = mybir.SyncInfo(
                        on_wait=[waits[-1]], on_update=list(si.on_update))
                new_list.append(inst)
            if changed:
                bb.instructions[:] = new_list
    return n_new


def _build_nc(collective=True):
    nc = bass.Bass()

    # ---- per-core DRAM I/O ----
    # vpk slab row = 256 low bytes + 64 crumb-packed high bytes
    vpk = nc.dram_tensor("vpk", [NPT, 16, 128, 320], U8, kind="ExternalInput")
    w2sh = nc.dram_tensor("w2sh", [64, NC2], BF16, kind="ExternalInput")
    misc = nc.dram_tensor("misc", [1, MISC_LEN], F32, kind="ExternalInput")
    u_out = nc.dram_tensor("u_out", [B, C_OUT, P], I8, kind="ExternalOutput")
    uinv = nc.dram_tensor("uinv", [B, 1], F32, kind="ExternalOutput")

    with tile.TileContext(nc) as tc:
        with tc.tile_pool(name="wpool", bufs=1) as wpool, \
             tc.tile_pool(name="dram", bufs=1, space="DRAM") as dram:

            # ---- stage 0: params into SBUF; AllGather w2 shards ----
            w2g = dram.tile([N_CORES * 64, NC2], BF16, addr_space="Shared")
            if collective:
                w2stage = dram.tile([64, NC2], BF16)
                nc.sync.dma_start(w2stage[:], w2sh[:])
                nc.gpsimd.collective_compute(
                    "AllGather", ALU.bypass,
                    replica_groups=[list(range(N_CORES))],
                    ins=[w2stage[:].opt()], outs=[w2g[:].opt()])
            else:
                nc.sync.dma_start(w2g[0:64, :], w2sh[:])

            coords_sb = wpool.tile([2, P], F32)
            nc.sync.dma_start(
                coords_sb[:],
                misc[0:1, OFF_COORDS:OFF_COORDS + 2 * P]
                .rearrange("o (p q) -> (o p) q", p=2))
            w1_psi_sb = wpool.tile([2, HID], F32)
            nc.sync.dma_start(
                w1_psi_sb[:],
                misc[0:1, OFF_W1PSI:OFF_W1PSI + 2 * HID]
                .rearrange("o (p q) -> (o p) q", p=2))
            w1_phi_sb = wpool.tile([2, HID], F32)
            nc.sync.dma_start(
                w1_phi_sb[:],
                misc[0:1, OFF_W1PHI:OFF_W1PHI + 2 * HID]
                .rearrange("o (p q) -> (o p) q", p=2))
            b1_psi_sb = wpool.tile([128, 2], F32)
            nc.sync.dma_start(
                b1_psi_sb[:],
                misc[0:1, OFF_B1PSI:OFF_B1PSI + 256]
                .rearrange("o (p q) -> (o p) q", p=128))
            b1_phi_sb = wpool.tile([128, 2], F32)
            nc.sync.dma_start(
                b1_phi_sb[:],
                misc[0:1, OFF_B1PHI:OFF_B1PHI + 256]
                .rearrange("o (p q) -> (o p) q", p=128))
            b2_psi_rep = wpool.tile([128, NC2], F32)
            nc.sync.dma_start(
                b2_psi_rep[:],
                misc[0:1, OFF_B2PSI:OFF_B2PSI + NC2].partition_broadcast(128))
            b2_phi_sb = wpool.tile([128, NC2 // 128], F32)
            nc.sync.dma_start(
                b2_phi_sb[:],
                misc[0:1, OFF_B2PHI:OFF_B2PHI + NC2]
                .rearrange("o (p q) -> (o p) q", p=128))
            svdx2_sb = wpool.tile([64, 1], F32)
            nc.sync.dma_start(
                svdx2_sb[:],
                misc[0:1, OFF_SVDX2:OFF_SVDX2 + 1].partition_broadcast(64))

            # w2 full tiles (bf16), per hid-half k: partition p <-> hid 128k+p
            w2_psi_sb = [wpool.tile([128, NC2], BF16, name=f"w2psi{k}",
                                    tag=f"w2psi{k}") for k in range(2)]
            w2_phi_sb = [wpool.tile([128, NC2], BF16, name=f"w2phi{k}",
                                    tag=f"w2phi{k}") for k in range(2)]
            for k in range(2):
                for m in range(4):
                    c = 4 * k + m
                    nc.sync.dma_start(w2_psi_sb[k][32 * m:32 * (m + 1), :],
                                      w2g[64 * c:64 * c + 32, :])
                    nc.sync.dma_start(w2_phi_sb[k][32 * m:32 * (m + 1), :],
                                      w2g[64 * c + 32:64 * (c + 1), :])

            # ---- stage A: hidden layers H_T = gelu(W1.T @ X^T + b1), bf16
            ht_psi = [wpool.tile([128, P], BF16, name=f"ht_psi{m}",
                                 tag=f"ht_psi{m}") for m in range(2)]
            ht_phi = [wpool.tile([128, P], BF16, name=f"ht_phi{m}",
                                 tag=f"ht_phi{m}") for m in range(2)]
            with tc.tile_pool(name="psumA", bufs=2, space="PSUM") as psumA:
                for (w1sb, b1sb, hts) in ((w1_psi_sb, b1_psi_sb, ht_psi),
                                          (w1_phi_sb, b1_phi_sb, ht_phi)):
                    for m in range(2):
                        ph = psumA.tile([128, P], F32, tag="ph")
                        for n in range(P // 512):
                            nc.tensor.matmul(
                                ph[:, 512 * n:512 * (n + 1)],
                                w1sb[:, 128 * m:128 * (m + 1)],
                                coords_sb[:, 512 * n:512 * (n + 1)],
                                start=True, stop=True)
                        nc.scalar.activation(
                            hts[m][:], ph[:], AF.Gelu,
                            bias=b1sb[:, m:m + 1], scale=1.0)

            # ---- stage B: v unpack + psi tiles + step-2 contraction ----
            ar_in = dram.tile([RANK, B], F32)
            ar_out = dram.tile([RANK, B], F32, addr_space="Shared")
            with tc.tile_pool(name="psumL2", bufs=1, space="PSUM") as psumL2, \
                 tc.tile_pool(name="bpool", bufs=2) as bpool, \
                 tc.tile_pool(name="psumB", bufs=1, space="PSUM") as psumB:
                l2acc = psumL2.tile([RANK, B], F32)
                for pt in range(NPT):
                    slab_pk = bpool.tile([128, 16 * 320], U8, tag="slab_pk")
                    nc.sync.dma_start(
                        slab_pk[:].rearrange("p (n f) -> p n f", f=320),
                        vpk[pt].rearrange("n p f -> p n f"))
                    pk3 = slab_pk[:].rearrange("p (n f) -> p n f", f=320)
                    slab_lo = pk3[:, :, 0:256]          # [128, 16, 256]
                    # crumb-packed high bits: col = n*256 + j*64 + 4*bq + t
                    hi4 = pk3[:, :, 256:320].rearrange(
                        "p n (j bq) -> p n j bq", j=4)  # [128, 16, 4, 16]
                    nib = bpool.tile([128, 16 * 256], U8, tag="nib")
                    nib5 = nib[:].rearrange("p (n j bq t) -> p n j bq t",
                                            n=16, j=4, bq=16, t=4)
                    for t in range(4):
                        nc.vector.tensor_scalar(
                            nib5[:, :, :, :, t], hi4, 2 * t, 3,
                            ALU.logical_shift_right, ALU.bitwise_and)
                    # v10 - 512 = (nib*256 - 512) + lo   (bf16)
                    vtmp = bpool.tile([128, 16 * 256], BF16, tag="vtmp")
                    nc.vector.tensor_scalar(
                        vtmp[:], nib[:], 256.0, -512.0, ALU.mult, ALU.add)
                    slab = bpool.tile([128, 16 * 256], BF16, tag="slab")
                    nc.vector.tensor_tensor(
                        slab[:].rearrange("p (n f) -> p n f", f=256),
                        vtmp[:].rearrange("p (n f) -> p n f", f=256),
                        slab_lo, ALU.add)

                    for half in range(2):
                        pp = psumB.tile([128, NC2 // 2], F32, tag="pp")
                        c0 = half * (NC2 // 2)
                        for k in range(2):
                            for n in range(NC2 // 2 // 512):
                                nc.tensor.matmul(
                                    pp[:, 512 * n:512 * (n + 1)],
                                    ht_psi[k][:, 128 * pt:128 * (pt + 1)],
                                    w2_psi_sb[k][:, c0 + 512 * n:c0 + 512 * (n + 1)],
                                    start=(k == 0), stop=(k == 1))
                        psit = bpool.tile([128, NC2 // 2], BF16, tag="psit")
                        nc.vector.tensor_add(psit[:], pp[:],
                                             b2_psi_rep[:, c0:c0 + NC2 // 2])
                        for il in range(32):
                            i = half * 32 + il
                            scol = (i // 4) * 256 + (i % 4) * 64
                            nc.tensor.matmul(
                                l2acc[:],
                                psit[:, 64 * il:64 * (il + 1)],
                                slab[:, scol:scol + 64],
                                start=(pt == 0 and i == 0),
                                stop=(pt == NPT - 1 and i == 63))

                # l2 finalize: scale by sv*dx^2 (per-call scalar), allreduce
                l2sb = bpool.tile([RANK, B], F32, tag="l2sb")
                nc.vector.tensor_scalar(
                    l2sb[:], l2acc[:], svdx2_sb[:, 0:1], None, ALU.mult)
                nc.sync.dma_start(ar_in[:], l2sb[:])

            if collective:
                nc.gpsimd.collective_compute(
                    "AllReduce", ALU.add,
                    replica_groups=[list(range(N_CORES))],
                    ins=[ar_in[:].opt()], outs=[ar_out[:].opt()])
            else:
                nc.sync.dma_start(ar_out[:], ar_in[:])
            l2dup = wpool.tile([128, B], F32)
            nc.sync.dma_start(l2dup[0:64, :], ar_out[:])
            nc.sync.dma_start(l2dup[64:128, :], ar_out[:])
            l2r = wpool.tile([128, B], F32R)
            nc.vector.tensor_copy(l2r[:], l2dup[:])

            # ---- stage C: phi tiles + step-4 expansion + int8 quantize ----
            uscr = dram.tile([B, C_OUT * P], F32)
            maxacc = wpool.tile([B, 1], F32)
            nc.vector.memset(maxacc[:], 1e-30)
            with tc.tile_pool(name="cpool", bufs=2) as cpool, \
                 tc.tile_pool(name="psumC", bufs=1, space="PSUM") as psumC, \
                 tc.tile_pool(name="psumU", bufs=4, space="PSUM") as psumU:
                for ct in range(NC2 // 128):   # 32 c'-tiles, 2 o-values each
                    pc = psumC.tile([128, P], F32, tag="pc")
                    for k in range(2):
                        for n in range(P // 512):
                            nc.tensor.matmul(
                                pc[:, 512 * n:512 * (n + 1)],
                                w2_phi_sb[k][:, 128 * ct:128 * (ct + 1)],
                                ht_phi[k][:, 512 * n:512 * (n + 1)],
                                start=(k == 0), stop=(k == 1))
                    phit = cpool.tile([128, P], F32R, tag="phit")
                    nc.scalar.activation(phit[:], pc[:], AF.Identity,
                                         bias=b2_phi_sb[:, ct:ct + 1], scale=1.0)
                    for oh in range(2):
                        o = 2 * ct + oh
                        ust = cpool.tile([B, P], F32, tag="ust")
                        for n in range(P // 512):
                            pu = psumU.tile([B, 512], F32, tag="pu")
                            nc.tensor.matmul(
                                pu[:],
                                l2r[64 * oh:64 * (oh + 1), :],
                                phit[64 * oh:64 * (oh + 1), 512 * n:512 * (n + 1)],
                                start=True, stop=True)
                            if n % 2 == 0:
                                nc.vector.tensor_copy(
                                    ust[:, 512 * n:512 * (n + 1)], pu[:])
                            else:
                                nc.scalar.activation(
                                    ust[:, 512 * n:512 * (n + 1)], pu[:], AF.Copy)
                        # per-b (partition) running abs-max
                        mx = cpool.tile([B, 1], F32, tag="mx")
                        nc.vector.tensor_reduce(
                            mx[:], ust[:], mybir.AxisListType.X, ALU.max,
                            apply_absolute_value=True)
                        nc.vector.tensor_tensor(
                            maxacc[:], maxacc[:], mx[:], ALU.max)
                        nc.sync.dma_start(uscr[:, P * o:P * (o + 1)], ust[:])

            # inv = 126 / max ; ship to host (decode divides by inv)
            rcp = wpool.tile([B, 1], F32)
            nc.vector.reciprocal(rcp[:], maxacc[:])
            inv_col = wpool.tile([B, 1], F32)
            nc.vector.tensor_scalar_mul(inv_col[:], rcp[:], 126.0)
            nc.sync.dma_start(uinv[:], inv_col[:])

            # pass 2: rescale spill to int8 (round-to-nearest via +-0.5)
            with tc.tile_pool(name="qpool", bufs=3) as qpool:
                for o in range(C_OUT):
                    us = qpool.tile([B, P], F32, tag="us")
                    nc.sync.dma_start(us[:], uscr[:, P * o:P * (o + 1)])
                    off = qpool.tile([B, P], F32, tag="off")
                    nc.vector.tensor_scalar(
                        off[:], us[:], 0.0, -0.5, ALU.is_ge, ALU.add)
                    y = qpool.tile([B, P], F32, tag="y")
                    nc.vector.tensor_scalar(
                        y[:], us[:], inv_col[:, 0:1], None, ALU.mult)
                    q = qpool.tile([B, P], I8, tag="q")
                    nc.vector.tensor_tensor(q[:], y[:], off[:], ALU.add)
                    nc.sync.dma_start(u_out[:, o, :], q[:])

    _split_multi_waits(nc)
    return nc


class _InMaps(list):
    """Per-core input dicts (run_bass_kernel_spmd-compatible) that also
    carry the stacked global arrays used by the cached executor."""
    globals: dict


def _prep_inputs(v, coords, psi_w1, psi_b1, psi_w2, psi_b2,
                 phi_w1, phi_b1, phi_w2, phi_b2):
    v = np.asarray(v, dtype=np.float32)
    coords = np.asarray(coords, dtype=np.float32)
    bf = ml_dtypes.bfloat16

    # column-permuted MLP2 weights: psi -> i-major (c' = i*64+r),
    # phi -> o-major (c' = o*64+r)
    w2p_psi = np.ascontiguousarray(
        np.asarray(psi_w2, np.float32).reshape(HID, RANK, C_IN)
        .transpose(0, 2, 1).reshape(HID, NC2)).astype(bf)
    b2p_psi = np.asarray(psi_b2, np.float32).reshape(RANK, C_IN).T.reshape(-1)
    w2p_phi = np.ascontiguousarray(
        np.asarray(phi_w2, np.float32).reshape(HID, RANK, C_OUT)
        .transpose(0, 2, 1).reshape(HID, NC2)).astype(bf)
    # b2_phi laid out [128, 32]: entry (p, t) = b2p[t*128 + p], c' o-major
    b2p_phi = np.asarray(phi_b2, np.float32).reshape(RANK, C_OUT).T \
        .reshape(32, 128).T

    w1_psi = np.asarray(psi_w1, np.float32)
    w1_phi = np.asarray(phi_w1, np.float32)
    b1_psi = np.asarray(psi_b1, np.float32).reshape(2, 128).T
    b1_phi = np.asarray(phi_b1, np.float32).reshape(2, 128).T

    # 10-bit fixed-point planes of v in slab layout
    # [H, 16, 128, 4, B]: (h, n, w, j, b) with i = 4n + j
    sv = max(float(np.abs(v).max()), 1e-30) / 511.0
    dx = float(coords[0, 1, 0] - coords[0, 0, 0])  # as the reference defines
    svdx2 = sv * dx * dx
    vt = v.transpose(2, 1, 3, 0)                       # [H, i, w, b]
    vt = vt.reshape(H, 16, 4, 128, B).transpose(0, 1, 3, 2, 4)
    q10 = np.clip(np.rint(vt * (1.0 / sv)), -511, 511).astype(np.int16)
    q10 += 512                                         # [1, 1023]
    lo = (q10 & 255).astype(np.uint8).reshape(N_CORES, NPT, 16, 128, 256)
    hi = (q10 >> 8).astype(np.uint8).reshape(N_CORES, NPT, 16, 128, 4, 16, 4)
    hi_pk = (hi[..., 0] | (hi[..., 1] << 2) | (hi[..., 2] << 4)
             | (hi[..., 3] << 6)).reshape(N_CORES, NPT, 16, 128, 64)
    vpk_g = np.concatenate([lo, hi_pk], axis=-1)       # [8, 16, 16, 128, 320]

    # w2 shards: core c gets psi rows [32c:32c+32] then phi rows
    w2sh_g = np.empty((N_CORES, 64, NC2), bf)
    for c in range(N_CORES):
        w2sh_g[c, 0:32] = w2p_psi[32 * c:32 * (c + 1)]
        w2sh_g[c, 32:64] = w2p_phi[32 * c:32 * (c + 1)]

    # misc param block per core
    misc_g = np.empty((N_CORES, 1, MISC_LEN), np.float32)
    shared = np.empty(MISC_LEN - 2 * P, np.float32)
    shared[OFF_W1PSI - 2 * P:OFF_W1PSI - 2 * P + 512] = w1_psi.reshape(-1)
    shared[OFF_W1PHI - 2 * P:OFF_W1PHI - 2 * P + 512] = w1_phi.reshape(-1)
    shared[OFF_B1PSI - 2 * P:OFF_B1PSI - 2 * P + 256] = b1_psi.reshape(-1)
    shared[OFF_B1PHI - 2 * P:OFF_B1PHI - 2 * P + 256] = b1_phi.reshape(-1)
    shared[OFF_B2PSI - 2 * P:OFF_B2PSI - 2 * P + NC2] = b2p_psi
    shared[OFF_B2PHI - 2 * P:OFF_B2PHI - 2 * P + NC2] = b2p_phi.reshape(-1)
    shared[OFF_SVDX2 - 2 * P] = svdx2
    for c in range(N_CORES):
        rows = slice(HL * c, HL * (c + 1))
        misc_g[c, 0, 0:2 * P] = coords[rows].reshape(P, 2).T.reshape(-1)
        misc_g[c, 0, 2 * P:] = shared

    in_maps = _InMaps(
        {"vpk": vpk_g[c], "w2sh": w2sh_g[c], "misc": misc_g[c]}
        for c in range(N_CORES))
    in_maps.globals = {
        "vpk": vpk_g.reshape(N_CORES * NPT, 16, 128, 320),
        "w2sh": w2sh_g.reshape(N_CORES * 64, NC2),
        "misc": misc_g.reshape(N_CORES, MISC_LEN),
    }
    return in_maps


def _get_executor():
    """Cached jit over the same _bass_exec_p custom call that
    run_bass_kernel_spmd lowers to under axon (see bass2jax.run_bass_via_pjrt),
    with output donation buffers kept device-resident across calls."""
    if "exec" in _CACHE:
        return _CACHE["exec"]
    import jax
    from jax.sharding import Mesh, PartitionSpec, NamedSharding
    from jax.experimental.shard_map import shard_map
    from concourse import bass2jax

    nc = _CACHE["nc"]
    bass2jax.install_neuronx_cc_hook()

    partition_name = (nc.partition_id_tensor.name
                      if nc.partition_id_tensor else None)
    in_names, out_names, out_avals, zero_outs = [], [], [], []
    for alloc in nc.m.functions[0].allocations:
        if not isinstance(alloc, mybir.MemoryLocationSet):
            continue
        name = alloc.memorylocations[0].name
        if alloc.kind == "ExternalInput":
            if name != partition_name:
                in_names.append(name)
        elif alloc.kind == "ExternalOutput":
            shape = tuple(alloc.tensor_shape)
            dtype = mybir.dt.np(alloc.dtype)
            out_names.append(name)
            out_avals.append(jax.core.ShapedArray(shape, dtype))
            zero_outs.append(np.zeros((N_CORES * shape[0], *shape[1:]), dtype))
    n_params = len(in_names)
    all_in_names = list(in_names) + list(out_names)
    if partition_name is not None:
        all_in_names.append(partition_name)

    devices = jax.devices()[:N_CORES]
    mesh = Mesh(np.asarray(devices), ("core",))
    sh = NamedSharding(mesh, PartitionSpec("core"))
    zdev = [jax.device_put(z, sh) for z in zero_outs]
    for z in zdev:
        z.block_until_ready()

    def _body(*args):
        operands = list(args)
        if partition_name is not None:
            operands.append(bass2jax.partition_id_tensor())
        outs = bass2jax._bass_exec_p.bind(
            *operands,
            out_avals=tuple(out_avals),
            in_names=tuple(all_in_names),
            out_names=tuple(out_names),
            lowering_input_output_aliases=(),
            sim_require_finite=True,
            sim_require_nnan=True,
            nc=nc,
        )
        return tuple(outs)

    n_outs = len(out_names)
    sharded = jax.jit(
        shard_map(_body, mesh=mesh,
                  in_specs=(PartitionSpec("core"),) * (n_params + n_outs),
                  out_specs=(PartitionSpec("core"),) * n_outs,
                  check_rep=False),
        keep_unused=True,
    )

    def run(globals_map):
        args = [globals_map[nm] for nm in in_names]
        outs = sharded(*args, *zdev)
        return dict(zip(out_names, outs))

    _CACHE["exec"] = run
    return run


def _decode(res):
    """int8 u + per-(core,b) inv -> full [B, C_OUT, H, W] f32."""
    inv = np.asarray(res["uinv"]).reshape(N_CORES, B)
    qg = np.asarray(res["u_out"]).reshape(N_CORES, B, C_OUT, HL, W)
    u = np.empty((B, C_OUT, H, W), np.float32)
    for c in range(N_CORES):
        sc = (1.0 / inv[c]).astype(np.float32)
        np.multiply(qg[c], sc[:, None, None, None],
                    out=u[:, :, HL * c:HL * (c + 1), :], casting="unsafe")
    return u


def _run_device(in_maps):
    """The device round-trip: upload wire tensors, run the Bass kernel on
    8 cores, fetch + decode the int8 output."""
    run = _get_executor()
    g = getattr(in_maps, "globals", None)
    if g is None:
        g = {
            "vpk": np.concatenate([m["vpk"] for m in in_maps], axis=0),
            "w2sh": np.concatenate([m["w2sh"] for m in in_maps], axis=0),
            "misc": np.concatenate([m["misc"] for m in in_maps], axis=0),
        }
    return _decode(run(g))


def kernel(**inputs):
    if "nc" not in _CACHE:
        _CACHE["nc"] = _build_nc()
    in_maps = _prep_inputs(**inputs)
    return _run_device(in_maps)


if __name__ == "__main__":
    pass
